# revision 1
# baseline (speedup 1.0000x reference)
"""BatchAllTripletLoss kernel for 8 Trainium2 NeuronCores.

Reference computation:
    pd = pairwise_euclidean(rep)                        # [512, 512]
    tl[a,p,k] = relu(pd[a,p] - pd[a,k] + 5.0) * mask    # [512, 512, 512]
    loss = sum(tl) / (count(tl > eps) + eps)

The mask (p!=a, k!=a, p!=k, label[p]==label[a], label[k]!=label[a])
collapses: label[p]==label[a] and label[k]!=label[a] imply p!=k and k!=a,
so valid triplets are exactly (anchor-positive pairs) x (k with a
different label).  With 64 labels over 512 rows there are only ~4100
(a,p) pairs, so instead of a dense [N,N,N] sweep each core processes its
anchors' pairs as rows of [128-pair, 512-k] tiles:

  per core (64 anchors):
    d[64,512]   = sqrt(relu(aug-matmul))            PE + DVE + ACT
    ym          = d + BIGM*same_label               DVE
    per pair-tile t:
      Gym       = sel_t.T @ ym                      PE one-hot row gather
      x[p]      = sum_k (iota==pidx)*Gym            DVE; = d[a,p] + BIGM
      xp        = x + (margin - BIGM)               DVE
      S_t[p]    = sum_k relu(xp - Gym)              ACT accum
      C_t[p]    = sum_k (Gym < xp)                  DVE accum
    out[1,2*Tp] = ones.T @ [S | C]                  PE partition sum

All matmuls run in float32r (single-pass fp32, ~2^-13 relative rounding;
the one-hot gather then carries that rounding into d).  BIGM = 128 both
masks out same-label k columns (xp <= ~35 << 128 so relu/count give
exactly 0) and carries the bias through the gather; the combined
rounding is ~1e-2 absolute per term, mean-zero, ~1e-4 on the final sums.
rep arrives both row-major (for the row-norm accumulates) and
host-transposed (pure layout permutation) so no PE transposes are
needed.  Anchors are block-sharded 64 per core; the 8 partial
(sum, count) pairs are reduced on the host (the all-reduce of the
sharding hint).  Host-side prep is integer/mask/layout logic only; all
float arithmetic runs on device.
"""

import ml_dtypes
import numpy as np

import concourse.bass as bass
import concourse.tile as tile
from concourse import bacc, mybir
from concourse.bass_utils import run_bass_kernel_spmd
from concourse.vector_clock import ScopedClock


_orig_aeb = bass.Bass.all_engine_barrier


def _skip_const_barrier(self, *, sem_only=False):
    if not getattr(self, "_aeb_skipped_once", False):
        self._aeb_skipped_once = True
        return
    return _orig_aeb(self, sem_only=sem_only)


def _cheap_drain_and_barrier(self, tick_clock, wait_clock):
    """Exit protocol with sequencer-only barriers: the SP drain already
    waits out every engine/DMA tick of the tile clock, so the per-engine
    pipeline drains of the stock double butterfly are redundant here."""
    drain_inst = self.nc.sync.drain()
    wait_clock.add_sem_waits(
        drain_inst.ins, ScopedClock({None: tick_clock.global_clock})
    )
    self.nc.all_engine_barrier(sem_only=True)
    popped = self.nc._tile_sem_poison_stack.pop()
    assert popped is self._sem_poison
    self.nc.clear_and_free_semaphores(list(self.sems.allocated().values()))
    self.nc.all_engine_barrier(sem_only=True)

F32 = mybir.dt.float32
F32R = mybir.dt.float32r
AF = mybir.ActivationFunctionType
OP = mybir.AluOpType

N = 512          # rows
D = 256          # embedding dim
NCORES = 8
A = N // NCORES  # anchors per core
MARGIN = 5.0
EPS = 1e-16
BIG = 1e30       # pad-pair kill value
BIGM = 128.0     # same-label mask / bias carrier (power of two)

_cache = {}


def _build(Tp: int):
    """Build the (uniform, SPMD) per-core Bass program for Tp pair tiles."""
    tile.TileContext._drain_and_barrier = _cheap_drain_and_barrier
    bass.Bass.all_engine_barrier = _skip_const_barrier
    nc = bacc.Bacc(None, target_bir_lowering=False, num_swdge_queues=2)

    rept_d = nc.declare_dram_parameter("rept", [128, 2, N], F32, isOutput=False)
    repa_d = nc.declare_dram_parameter("repa", [A, D], F32, isOutput=False)
    repat_d = nc.declare_dram_parameter("repat", [128, 2, A], F32, isOutput=False)
    bigm_d = nc.declare_dram_parameter("bigm", [A, N], mybir.dt.float8e4, isOutput=False)
    sel_d = nc.declare_dram_parameter("sel", [A, Tp * 128], mybir.dt.float8e4, isOutput=False)
    pm_d = nc.declare_dram_parameter("pm", [128, 2 * Tp], F32, isOutput=False)
    out_d = nc.declare_dram_parameter("out", [1, 2 * Tp], F32, isOutput=True)

    with tile.TileContext(nc) as tc:
        with (
            tc.tile_pool(name="singles", bufs=1) as sg,
            tc.tile_pool(name="scr", bufs=2) as scr,
            tc.tile_pool(name="xs", bufs=3) as xs,
            tc.tile_pool(name="ppf", bufs=1, space="PSUM") as ppf,
            tc.tile_pool(name="ppg", bufs=4, space="PSUM") as ppg,
            tc.tile_pool(name="ppd", bufs=1, space="PSUM") as ppd,
        ):
            iota_f = sg.tile([128, N], F32)
            nc.gpsimd.iota(
                iota_f[:], [[1, N]], channel_multiplier=0,
                allow_small_or_imprecise_dtypes=True,
            )
            ones = sg.tile([128, 1], F32)
            nc.vector.memset(ones[:], 1.0)
            onesr = sg.tile([128, 1], F32R)
            nc.vector.tensor_copy(onesr[:], ones[:])
            ones1 = sg.tile([1, A], F32)
            nc.vector.memset(ones1[:], 1.0)
            ones1r = sg.tile([1, A], F32R)
            nc.vector.tensor_copy(ones1r[:], ones1[:])
            dmy = sg.tile([1, 1], F32)
            nc.scalar.activation(dmy[:], ones[0:1, :], AF.Sqrt, bias=ones[0:1, :])

            # input loads, spread across the two HWDGE queues; rep first
            # (the row-norm chain below is the longest dependency chain)
            rept_s = sg.tile([128, 2, N], F32)     # rept[p, c, j] = rep[j, c*128+p]
            for q in range(4):
                eng = nc.sync if q % 2 == 0 else nc.scalar
                eng.dma_start(
                    rept_s[:, q // 2, (q % 2) * 256:(q % 2) * 256 + 256],
                    rept_d[:, q // 2, (q % 2) * 256:(q % 2) * 256 + 256],
                )
            repat_s = sg.tile([128, 2, A], F32)    # repat[p, c, a] = repa[a, c*128+p]
            nc.gpsimd.dma_start(repat_s[:], repat_d[:])
            repa_s = sg.tile([A, D], F32)
            nc.gpsimd.dma_start(repa_s[:], repa_d[:])
            bigm_s = sg.tile([A, N], mybir.dt.float8e4)
            nc.gpsimd.dma_start(bigm_s[:], bigm_d[:])
            sel_s = sg.tile([A, Tp * 128], mybir.dt.float8e4)
            nc.gpsimd.dma_start(sel_s[:], sel_d[:])
            pm_s = sg.tile([128, 2 * Tp], F32)     # [:, :Tp] pidx, [:, Tp:] margin
            nc.gpsimd.dma_start(pm_s[:], pm_d[:])

            # float32r operand copies (PE consumes pre-rounded data), per
            # chunk so each overlaps the other chunk's DMA
            reptr = sg.tile([128, 2, N], F32R)
            for c in range(2):
                nc.vector.tensor_copy(reptr[:, c, :], rept_s[:, c, :])
            negTa = sg.tile([128, 2, A], F32R)
            nc.vector.tensor_scalar_mul(negTa[:], repat_s[:], -2.0)

            # d2[a, j] = sq_a + sq_j - 2*dot: start the big -2*dot matmuls as
            # soon as the casts land; the sq_j rank-1 terms join the group last
            d2_p = ppd.tile([A, N], F32, tag="d2")
            nc.tensor.matmul(d2_p[:], negTa[:, 0, :], reptr[:, 0, :],
                             start=True, stop=False, skip_group_check=True)
            nc.tensor.matmul(d2_p[:], negTa[:, 1, :], reptr[:, 1, :],
                             start=False, stop=False, skip_group_check=True)

            # sq_row[1, j] = ||rep_j||^2 = ones.T @ (rept * rept)
            sqsq = sg.tile([128, 2, N], F32R)
            for c in range(2):
                nc.vector.tensor_mul(sqsq[:, c, :], rept_s[:, c, :], rept_s[:, c, :])
            sqrow_p = ppf.tile([1, N], F32, tag="fin")
            nc.tensor.matmul(sqrow_p[:], onesr[:], sqsq[:, 0, :], start=True,
                             stop=False, skip_group_check=True)
            nc.tensor.matmul(sqrow_p[:], onesr[:], sqsq[:, 1, :], start=False,
                             stop=True, skip_group_check=True)
            sqrowr = sg.tile([1, N], F32R)
            nc.vector.tensor_copy(sqrowr[:], sqrow_p[:])
            nc.tensor.matmul(d2_p[:], ones1r[:], sqrowr[:], start=False, stop=True,
                             skip_group_check=True)

            # sq_anch[64,1] = ||rep_a||^2
            sqa_scr = scr.tile([A, D], F32, tag="sqa")
            sqanch = sg.tile([A, 1], F32)
            nc.vector.scalar_tensor_tensor(
                out=sqa_scr[:], in0=repa_s[:], scalar=1.0, in1=repa_s[:],
                op0=OP.mult, op1=OP.mult, accum_out=sqanch[:],
            )

            selr = sg.tile([A, Tp * 128], F32R)
            nc.vector.tensor_copy(selr[:], sel_s[:])

            # ym = sqrt(d2 + 0.25) + BIGM*same: the +0.25 keeps the (masked)
            # diagonal's rounding noise out of sqrt's domain; its effect on
            # d_ap - d_ak cancels to ~5e-4
            sqanchb = xs.tile([A, 1], F32, tag="sqb")
            nc.vector.tensor_scalar(sqanchb[:], sqanch[:], 0.25, None, OP.add)
            dtmp = scr.tile([A, N], F32, tag="dtmp")
            nc.scalar.activation(dtmp[:], d2_p[:], AF.Sqrt, bias=sqanchb[:])
            ym = sg.tile([A, N], F32R)
            nc.vector.tensor_add(ym[:], bigm_s[:], dtmp[:])

            # pair tiles
            SC = sg.tile([128, 2 * Tp], F32)
            nc.vector.memset(SC[:], 0.0)
            relbig = sg.tile([128, Tp, N], F32)
            for t in range(Tp):
                gy = ppg.tile([128, N], F32, tag="gy")
                nc.tensor.matmul(gy[:], selr[:, t * 128:(t + 1) * 128], ym[:],
                                 start=True, stop=True)

                stt = scr.tile([128, N], F32, tag="stt")
                xv = xs.tile([128, 1], F32, tag="xv")
                nc.vector.scalar_tensor_tensor(
                    out=stt[:], in0=iota_f[:], scalar=pm_s[:, t:t + 1], in1=gy[:],
                    op0=OP.is_equal, op1=OP.mult, accum_out=xv[:],
                )
                xp = xs.tile([128, 1], F32, tag="xp")
                nc.vector.tensor_scalar(
                    xp[:], xv[:], pm_s[:, Tp + t:Tp + t + 1], None, OP.add
                )

                nc.scalar.activation(
                    relbig[:, t, :], gy[:], AF.Relu, bias=xp[:], scale=-1.0,
                    accum_out=SC[:, t:t + 1],
                )

            # counts: relu output is positive exactly where a triplet is
            # positive, so two wide scans replace five per-tile ones
            h = (Tp + 1) // 2
            nc.vector.tensor_scalar(
                relbig[:, 0:h, :], relbig[:, 0:h, :], 0.0, 0.0, OP.is_gt, OP.add,
                accum_out=SC[:, Tp:Tp + 1],
            )
            if Tp > h:
                nc.vector.tensor_scalar(
                    relbig[:, h:Tp, :], relbig[:, h:Tp, :], 0.0, 0.0,
                    OP.is_gt, OP.add,
                    accum_out=SC[:, Tp + 1:Tp + 2],
                )

            # partition-sum S and C columns -> [1, 2*Tp]
            fin_p = ppf.tile([1, 2 * Tp], F32, tag="fin")
            nc.tensor.matmul(fin_p[:], ones[:], SC[:], start=True, stop=True)
            outsb = sg.tile([1, 2 * Tp], F32)
            nc.vector.tensor_copy(outsb[:], fin_p[:])
            nc.sync.dma_start(out_d[:], outsb[:])

    nc.finalize()
    return nc


def _prep(rep: np.ndarray, labels: np.ndarray):
    """Host-side integer/mask/layout prep: shard anchors, enumerate pairs."""
    rep = np.ascontiguousarray(np.asarray(rep, dtype=np.float32))
    labels = np.asarray(labels)
    same = labels[:, None] == labels[None, :]

    # rep.T packed [128, 2, N]: rept[p, c, j] = rep[j, c*128 + p]
    rept = np.ascontiguousarray(
        rep.T.reshape(2, 128, N).transpose(1, 0, 2)
    )

    pairs = []
    for c in range(NCORES):
        base = c * A
        prs = [
            (j, p)
            for j in range(A)
            for p in np.nonzero(same[base + j])[0]
            if p != base + j
        ]
        pairs.append(prs)
    Tp = max(1, max((len(p) + 127) // 128 for p in pairs))

    in_maps = []
    for c in range(NCORES):
        base = c * A
        repa = rep[base:base + A]
        repat = np.ascontiguousarray(
            repa.T.reshape(2, 128, A).transpose(1, 0, 2)
        )
        bigm = np.where(same[base:base + A], BIGM, 0.0).astype(ml_dtypes.float8_e4m3)
        sel = np.zeros((A, Tp * 128), ml_dtypes.float8_e4m3)
        pm = np.zeros((128, 2 * Tp), np.float32)
        pm[:, Tp:] = -BIG
        for i, (j, p) in enumerate(pairs[c]):
            t, r = divmod(i, 128)
            sel[j, i] = 1.0
            pm[r, t] = p
            pm[r, Tp + t] = MARGIN - BIGM
        in_maps.append({
            "rept": rept,
            "repa": repa,
            "repat": repat,
            "bigm": bigm,
            "sel": sel,
            "pm": pm,
        })
    return Tp, in_maps


def _run(rep, labels, trace=False):
    Tp, in_maps = _prep(rep, labels)
    if Tp not in _cache:
        _cache[Tp] = _build(Tp)
    nc = _cache[Tp]
    res = run_bass_kernel_spmd(nc, in_maps, list(range(NCORES)), trace=trace)
    outs = np.stack([res.results[c]["out"][0] for c in range(NCORES)])  # [8, 2*Tp]
    S = float(outs[:, :Tp].sum())
    C = float(outs[:, Tp:].sum())
    loss = np.float32(S / (C + EPS))
    return np.asarray(loss, dtype=np.float32), res


def kernel(rep, labels):
    loss, _ = _run(rep, labels, trace=False)
    return loss



# revision 11
# speedup vs baseline: 1.1130x; 1.1130x over previous
"""BatchAllTripletLoss kernel for 8 Trainium2 NeuronCores.

Reference computation:
    pd = pairwise_euclidean(rep)                        # [512, 512]
    tl[a,p,k] = relu(pd[a,p] - pd[a,k] + 5.0) * mask    # [512, 512, 512]
    loss = sum(tl) / (count(tl > eps) + eps)

The mask (p!=a, k!=a, p!=k, label[p]==label[a], label[k]!=label[a])
collapses: valid triplets are exactly (anchor-positive pairs) x (k with a
different label).  With 64 labels over 512 rows there are only ~4500
(a,p) pairs, so each core processes its 64 anchors' pairs as rows of
[128-pair, 512-k] tiles:

  per core:
    d[64,512]  = sqrt(d2-matmul-group)                PE + ACT
    ym         = d + BIGM*same_label                  DVE (bf16)
    xp[i]      = sqrt(sum_c (rep_a - rep_p)^2) + m    DVE diff/sq, PE
                 column sums, ACT sqrt, PE 1-row transpose matmuls
                 (host-gathered anchor/positive columns AP/PP)
    per tile:  gy = sel_t.T @ ym (PE one-hot row gather)
               relu(xp - gy) -> relbig (ACT, bf16)
    S, C       = two wide DVE scans each (sum / is_gt) over relbig
    out[1,4]   = ones.T @ [S_h1 S_h2 C_h1 C_h2]       PE partition sum

All device data is bf16 (fp32 accumulation in PSUM / accumulators); the
rounding is mean-zero across ~1M triplets and lands ~1e-3 relative on
the loss, well inside the 2e-2 gate.  BIGM=512 masks same-label columns
(xp <= ~50 << 512).  Dead pair slots get xp = -1e30 via the mpad row.
Host-side prep is layout/gather/cast only (plus an exact *-2 on the
anchor transpose); all float arithmetic runs on device.  Anchors are
block-sharded 64 per core; the 8 partial (S, C) pairs are reduced on
the host (the all-reduce of the sharding hint).

Overhead trims: the stock Tile exit protocol is replaced with a
sequencer-drain + two barriers + one semaphore RANGE_CLEAR; gpsimd
issues no DMAs so the qPoolDynamic queue group is pruned from the NEFF
(fewer runtime rings to set up / tear down); the activation-table
chooser is steered so Sqrt and Relu share one table load.
"""

import ml_dtypes
import numpy as np

import concourse.bass as bass
import concourse.tile as tile
from concourse import bacc, mybir
from concourse.bass_utils import run_bass_kernel_spmd
from concourse.vector_clock import ScopedClock


_orig_aeb = bass.Bass.all_engine_barrier


def _skip_const_barrier(self, *, sem_only=False):
    if not getattr(self, "_aeb_skipped_once", False):
        self._aeb_skipped_once = True
        return
    return _orig_aeb(self, sem_only=sem_only)


def _cheap_drain_and_barrier(self, tick_clock, wait_clock):
    """Exit protocol with sequencer-only barriers: the SP drain already
    waits out every engine/DMA tick of the tile clock, so the per-engine
    pipeline drains of the stock double butterfly are redundant here."""
    drain_inst = self.nc.sync.drain()
    wait_clock.add_sem_waits(
        drain_inst.ins, ScopedClock({None: tick_clock.global_clock})
    )
    self.nc.all_engine_barrier(sem_only=True)
    popped = self.nc._tile_sem_poison_stack.pop()
    assert popped is self._sem_poison
    self.nc.clear_and_free_semaphores(list(self.sems.allocated().values()))
    self.nc.all_engine_barrier(sem_only=True)


_orig_gat = bacc.get_activation_tables
_AF = mybir.ActivationFunctionType


def _sqrt_set_only(arch):
    """Strip Sqrt/Relu from every set but sqrt_and_others so the table
    chooser lands both on one set (one ACT_TABLE_LOAD instead of two).
    Keys/order preserved so act_func_set_id indexing is unchanged."""
    t = _orig_gat(arch)
    out = {}
    for k, v in t.items():
        if k == "sqrt_and_others":
            out[k] = v
        else:
            out[k] = {f for f in v if f not in (_AF.Sqrt, _AF.Relu)}
    return out


bacc.get_activation_tables = _sqrt_set_only

F32 = mybir.dt.float32
BF16 = mybir.dt.bfloat16
AF = mybir.ActivationFunctionType
OP = mybir.AluOpType

N = 512          # rows
D = 256          # embedding dim
NCORES = 8
A = N // NCORES  # anchors per core
MARGIN = 5.0
EPS = 1e-16
BIGM = 512.0     # same-label mask (power of two, exact in bf16)
DEAD = -1e30     # dead pair-slot kill value

_cache = {}


def _build(Tp: int):
    """Build the (uniform, SPMD) per-core Bass program for Tp pair tiles."""
    tile.TileContext._drain_and_barrier = _cheap_drain_and_barrier
    bass.Bass.all_engine_barrier = _skip_const_barrier
    nc = bacc.Bacc(None, target_bir_lowering=False)
    # gpsimd issues no DMAs in this kernel: drop its queue group so the
    # runtime has fewer rings to manage.
    nc.m.queues = [q for q in nc.m.queues if not q.name.startswith("qPoolDynamic")]

    P = Tp * 128
    # bf16 column layout of the packed input (one DRAM tensor):
    O_REPT = 0                  # [128, 2*512]  rept[p, c*512+j] = rep[j, c*128+p]
    O_REPAT2 = O_REPT + 1024    # [128, 2*64]   -2 * rep[base+a, c*128+p]
    O_AP = O_REPAT2 + 128       # [128, 2*P]    rep[a_i, c*128+p]
    O_PP = O_AP + 2 * P         # [128, 2*P]    rep[p_i, c*128+p]
    O_X2 = O_PP + 2 * P         # [2, P]        part 0: ACT scratch, part 1: mpad
    O_SEL = O_X2 + P            # [64, P]       one-hot pair->anchor gather
    O_BIGM = O_SEL + P          # [64, 512]     BIGM * same_label
    O_REPA = O_BIGM + 512       # [64, 256]     rep[base+a, :] (row-major)
    COLS = O_REPA + 256

    W = 4                       # out cols: [S_h1, S_h2, C_h1, C_h2]

    pk_d = nc.declare_dram_parameter("pk", [128, COLS], BF16, isOutput=False)
    out_d = nc.declare_dram_parameter("out", [1, W], F32, isOutput=True)

    h1 = (Tp + 1) // 2          # S-scan split

    with tile.TileContext(nc) as tc:
        with (
            tc.tile_pool(name="singles", bufs=1) as sg,
            tc.tile_pool(name="ppd", bufs=1, space="PSUM") as ppd,
            tc.tile_pool(name="ppp", bufs=1, space="PSUM") as ppp,
            tc.tile_pool(name="ppx", bufs=1, space="PSUM") as ppx,
            tc.tile_pool(name="ppg", bufs=3, space="PSUM") as ppg,
            tc.tile_pool(name="ppf", bufs=1, space="PSUM") as ppf,
        ):
            pk = sg.tile([128, COLS], BF16)
            # input loads: sync queue carries rept/repat2 then sel/bigm/repa
            # then the 1-partition mpad row; scalar queue carries AP/PP.
            nc.sync.dma_start(pk[:, O_REPT:O_AP], pk_d[:, O_REPT:O_AP])
            nc.scalar.dma_start(pk[:, O_AP:O_X2], pk_d[:, O_AP:O_X2])
            nc.sync.dma_start(pk[:, O_SEL:COLS], pk_d[:, O_SEL:COLS])
            nc.sync.dma_start(pk[1:2, O_X2:O_SEL], pk_d[1:2, O_X2:O_SEL])

            ones_c = sg.tile([128, A], BF16)
            nc.vector.memset(ones_c[:], 1.0)
            ones_p = sg.tile([128, 1], BF16)
            nc.gpsimd.memset(ones_p[:], 1.0)
            ones2 = sg.tile([2, 1], BF16)
            nc.gpsimd.memset(ones2[:], 1.0)
            onesf = sg.tile([128, 1], F32)
            nc.gpsimd.memset(onesf[:], 1.0)

            # ---- main d2: sq_a + sq_j - 2 a.j for 64 anchors x 512 j ----
            sqsq = sg.tile([128, 1024], BF16)
            nc.vector.tensor_mul(
                sqsq[:], pk[:, O_REPT:O_REPAT2], pk[:, O_REPT:O_REPAT2]
            )
            d2_p = ppd.tile([A, N], F32, tag="d2")
            for c in range(2):
                nc.tensor.matmul(
                    d2_p[:],
                    pk[:, O_REPAT2 + c * A:O_REPAT2 + (c + 1) * A],
                    pk[:, O_REPT + c * 512:O_REPT + (c + 1) * 512],
                    start=(c == 0), stop=False, skip_group_check=True,
                )
            for c in range(2):
                nc.tensor.matmul(
                    d2_p[:], ones_c[:], sqsq[:, c * 512:(c + 1) * 512],
                    start=False, stop=(c == 1), skip_group_check=True,
                )

            # sq_anch[64,1] (repa rows live on partitions 0-63)
            sqa_scr = sg.tile([64, D], BF16)
            sqanch = sg.tile([A, 1], F32)
            nc.vector.scalar_tensor_tensor(
                out=sqa_scr[:], in0=pk[0:64, O_REPA:O_REPA + 256], scalar=1.0,
                in1=pk[0:64, O_REPA:O_REPA + 256],
                op0=OP.mult, op1=OP.mult, accum_out=sqanch[:],
            )
            sqanchb = sg.tile([A, 1], F32)
            nc.vector.tensor_scalar(sqanchb[:], sqanch[:], 0.25, None, OP.add)

            # ym = sqrt(d2 + 0.25) + BIGM*same  (the +0.25 keeps the masked
            # diagonal's accumulation-order noise out of sqrt's domain)
            dtmp = sg.tile([A, N], BF16)
            nc.scalar.activation(dtmp[:], d2_p[:], AF.Sqrt, bias=sqanchb[:])
            ym = sg.tile([A, N], BF16)
            nc.vector.tensor_add(ym[:], pk[0:64, O_BIGM:O_BIGM + 512], dtmp[:])

            # ---- pair distances: xp_i = sqrt(sum_c (AP-PP)^2) + mpad ----
            diff = sg.tile([128, 2 * P], BF16)
            nc.vector.tensor_tensor(
                diff[:], pk[:, O_AP:O_AP + 2 * P], pk[:, O_PP:O_PP + 2 * P],
                op=OP.subtract,
            )
            dsq = sg.tile([128, 2 * P], BF16)
            nc.vector.tensor_mul(dsq[:], diff[:], diff[:])
            pd2a = ppp.tile([1, 512], F32, tag="pd2a")
            for c in range(2):
                nc.tensor.matmul(
                    pd2a[:], ones_p[:], dsq[:, c * P:c * P + 512],
                    start=(c == 0), stop=(c == 1), skip_group_check=True,
                )
            nc.scalar.activation(pk[0:1, O_X2:O_X2 + 512], pd2a[:], AF.Sqrt)
            if P > 512:
                pd2b = ppp.tile([1, P - 512], F32, tag="pd2b")
                for c in range(2):
                    nc.tensor.matmul(
                        pd2b[:], ones_p[:], dsq[:, c * P + 512:c * P + P],
                        start=(c == 0), stop=(c == 1), skip_group_check=True,
                    )
                nc.scalar.activation(pk[0:1, O_X2 + 512:O_X2 + P], pd2b[:], AF.Sqrt)

            # transpose the [1, P] xp row into [128, Tp] columns via
            # 2-row matmuls against ones: out[r, t] = xv[t*128+r] + mpad[..]
            xp_p = ppx.tile([128, Tp], F32, tag="xpp")
            for t in range(Tp):
                nc.tensor.matmul(
                    xp_p[:, t:t + 1],
                    pk[0:2, O_X2 + t * 128:O_X2 + (t + 1) * 128],
                    ones2[:], start=True, stop=True,
                )
            xp_all = sg.tile([128, Tp], F32)
            nc.vector.tensor_copy(xp_all[:], xp_p[:])

            # ---- pair tiles: gather + relu ----
            SC = sg.tile([128, W], F32)
            relbig = sg.tile([128, Tp, N], BF16)
            for t in range(Tp):
                gy = ppg.tile([128, N], F32, tag="gy")
                nc.tensor.matmul(
                    gy[:], pk[0:64, O_SEL + t * 128:O_SEL + (t + 1) * 128],
                    ym[:], start=True, stop=True,
                )
                nc.scalar.activation(
                    relbig[:, t, :], gy[:], AF.Relu,
                    bias=xp_all[:, t:t + 1], scale=-1.0,
                )

            # S and C: two wide DVE scans per metric over relbig (bf16),
            # split in halves so the first half overlaps the later relus
            js1 = sg.tile([128, h1 * N], BF16)
            js2 = sg.tile([128, (Tp - h1) * N], BF16)
            nc.vector.tensor_scalar(
                js1[:], relbig[:, 0:h1, :], 0.0, 0.0, OP.add, OP.add,
                accum_out=SC[:, 0:1],
            )
            nc.vector.tensor_scalar(
                js1[:], relbig[:, 0:h1, :], 0.0, 0.0, OP.is_gt, OP.add,
                accum_out=SC[:, 2:3],
            )
            nc.vector.tensor_scalar(
                js2[:], relbig[:, h1:Tp, :], 0.0, 0.0, OP.add, OP.add,
                accum_out=SC[:, 1:2],
            )
            nc.vector.tensor_scalar(
                js2[:], relbig[:, h1:Tp, :], 0.0, 0.0, OP.is_gt, OP.add,
                accum_out=SC[:, 3:4],
            )

            # partition-sum the S and C columns -> [1, W]
            fin_p = ppf.tile([1, W], F32, tag="fin")
            nc.tensor.matmul(fin_p[:], onesf[:], SC[:], start=True, stop=True)
            outsb = sg.tile([1, W], F32)
            nc.scalar.copy(outsb[:], fin_p[:])
            nc.sync.dma_start(out_d[:], outsb[:])

    nc.finalize()
    return nc


def _prep(rep: np.ndarray, labels: np.ndarray):
    """Host-side layout/gather/cast prep: shard anchors, enumerate pairs."""
    rep = np.ascontiguousarray(np.asarray(rep, dtype=np.float32))
    repb = rep.astype(ml_dtypes.bfloat16)
    labels = np.asarray(labels)
    same = labels[:, None] == labels[None, :]

    # rep.T packed: rept[p, c*512 + j] = rep[j, c*128 + p]
    rept = np.ascontiguousarray(
        repb.T.reshape(2, 128, N).transpose(1, 0, 2).reshape(128, 1024)
    )

    pairs = []
    for c in range(NCORES):
        base = c * A
        prs = [
            (j, p)
            for j in range(A)
            for p in np.nonzero(same[base + j])[0]
            if p != base + j
        ]
        pairs.append(prs)
    Tp = max(1, max((len(p) + 127) // 128 for p in pairs))
    P = Tp * 128

    O_REPT = 0
    O_REPAT2 = 1024
    O_AP = O_REPAT2 + 128
    O_PP = O_AP + 2 * P
    O_X2 = O_PP + 2 * P
    O_SEL = O_X2 + P
    O_BIGM = O_SEL + P
    O_REPA = O_BIGM + 512
    COLS = O_REPA + 256

    in_maps = []
    for c in range(NCORES):
        base = c * A
        npair = len(pairs[c])
        a_idx = np.zeros(P, np.int64)
        p_idx = np.zeros(P, np.int64)
        for i, (j, p) in enumerate(pairs[c]):
            a_idx[i] = base + j
            p_idx[i] = p

        pk = np.zeros((128, COLS), ml_dtypes.bfloat16)
        pk[:, O_REPT:O_REPAT2] = rept
        # -2 * anchor transpose (exact scale)
        repa32 = rep[base:base + A]
        pk[:, O_REPAT2:O_AP] = np.ascontiguousarray(
            (-2.0 * repa32).T.reshape(2, 128, A).transpose(1, 0, 2).reshape(128, 2 * A)
        ).astype(ml_dtypes.bfloat16)
        # gathered anchor / positive columns (bf16 of rep, pure gather)
        gA = repb[a_idx].T.reshape(2, 128, P).transpose(1, 0, 2).reshape(128, 2 * P)
        gP = repb[p_idx].T.reshape(2, 128, P).transpose(1, 0, 2).reshape(128, 2 * P)
        pk[:, O_AP:O_PP] = gA
        pk[:, O_PP:O_X2] = gP
        # mpad row (partition 1 of the X2 region)
        mpad = np.full(P, DEAD, np.float32)
        mpad[:npair] = MARGIN
        pk[1, O_X2:O_SEL] = mpad.astype(ml_dtypes.bfloat16)
        # dead AP/PP slots are rep[0] - rep[0] = 0 -> xv 0, xp DEAD
        sel = np.zeros((A, P), ml_dtypes.bfloat16)
        for i, (j, p) in enumerate(pairs[c]):
            sel[j, i] = 1.0
        pk[0:64, O_SEL:O_BIGM] = sel
        pk[0:64, O_BIGM:O_REPA] = np.where(
            same[base:base + A], BIGM, 0.0
        ).astype(ml_dtypes.bfloat16)
        pk[0:64, O_REPA:COLS] = repb[base:base + A]
        in_maps.append({"pk": pk})
    return Tp, in_maps


def _run(rep, labels, trace=False):
    Tp, in_maps = _prep(rep, labels)
    if Tp not in _cache:
        _cache[Tp] = _build(Tp)
    nc = _cache[Tp]
    res = run_bass_kernel_spmd(nc, in_maps, list(range(NCORES)), trace=trace)
    outs = np.stack([res.results[c]["out"][0] for c in range(NCORES)])  # [8, 4]
    S = float(outs[:, 0:2].sum())
    C = float(outs[:, 2:4].sum())
    loss = np.float32(S / (C + EPS))
    return np.asarray(loss, dtype=np.float32), res


def kernel(rep, labels):
    loss, _ = _run(rep, labels, trace=False)
    return loss


# revision 12
# speedup vs baseline: 1.2604x; 1.1324x over previous
"""BatchAllTripletLoss kernel for 8 Trainium2 NeuronCores.

Reference computation:
    pd = pairwise_euclidean(rep)                        # [512, 512]
    tl[a,p,k] = relu(pd[a,p] - pd[a,k] + 5.0) * mask    # [512, 512, 512]
    loss = sum(tl) / (count(tl > eps) + eps)

The mask (p!=a, k!=a, p!=k, label[p]==label[a], label[k]!=label[a])
collapses: valid triplets are exactly (anchor-positive pairs) x (k with a
different label).  With 64 labels over 512 rows there are only ~4500
(a,p) pairs, so each core processes its 64 anchors' pairs as rows of
[128-pair, 512-k] tiles:

  per core:
    d[64,512]  = sqrt(d2-matmul-group)                  PE + ACT
    ym         = d + BIGM*same_label                    DVE (bf16)
    xp[128,Tp] = sqrt(sum_c (rep_a - rep_p)^2) + m      DVE diff + per-tile
                 square-accumulate, one ACT sqrt        (host-gathered
                 anchor/positive rows AP/PP, pair-major layout)
    per tile:  gy = sel_t.T @ ym (PE one-hot row gather)
               S_t = accum relu(xp - gy)                ACT (bf16 out)
               C_t = accum (gy < xp)                    DVE on gy
    out[1,10]  = ones.T @ [S_t | C_t]                   PE partition sum

All device data is bf16 (fp32 accumulation in PSUM / accumulators); the
rounding is mean-zero across ~1M triplets and lands ~1e-3 relative on
the loss, well inside the 2e-2 gate.  BIGM=512 masks same-label columns
(xp <= ~50 << 512).  Dead pair slots get xp = -1e30 via the host pmadd
row.  Host-side prep is layout/gather/cast only (plus an exact *-2 on
the anchor transpose); all float arithmetic runs on device.  Anchors
are block-sharded 64 per core; the 8 partial (S, C) pairs are reduced
on the host (the all-reduce of the sharding hint).

Overhead trims: the stock Tile exit protocol is replaced with a
sequencer-drain + two barriers + one semaphore RANGE_CLEAR; gpsimd
issues no DMAs so the qPoolDynamic queue group is pruned from the NEFF;
the activation-table chooser is steered so Sqrt and Relu share one
table load, and a dependency-free warmup sqrt pins that load to the
very start of the ACT stream (off the critical path).
"""

import ml_dtypes
import numpy as np

import concourse.bass as bass
import concourse.tile as tile
from concourse import bacc, mybir
from concourse.bass_utils import run_bass_kernel_spmd
from concourse.vector_clock import ScopedClock


_orig_aeb = bass.Bass.all_engine_barrier


def _skip_const_barrier(self, *, sem_only=False):
    if not getattr(self, "_aeb_skipped_once", False):
        self._aeb_skipped_once = True
        return
    return _orig_aeb(self, sem_only=sem_only)


def _cheap_drain_and_barrier(self, tick_clock, wait_clock):
    """Exit protocol with sequencer-only barriers: the SP drain already
    waits out every engine/DMA tick of the tile clock, so the per-engine
    pipeline drains of the stock double butterfly are redundant here."""
    drain_inst = self.nc.sync.drain()
    wait_clock.add_sem_waits(
        drain_inst.ins, ScopedClock({None: tick_clock.global_clock})
    )
    self.nc.all_engine_barrier(sem_only=True)
    popped = self.nc._tile_sem_poison_stack.pop()
    assert popped is self._sem_poison
    self.nc.clear_and_free_semaphores(list(self.sems.allocated().values()))
    self.nc.all_engine_barrier(sem_only=True)


_orig_gat = bacc.get_activation_tables
_AF = mybir.ActivationFunctionType


def _sqrt_set_only(arch):
    """Strip the functions this kernel uses from every set but
    sqrt_and_others so the table chooser lands them all on one set (one
    ACT_TABLE_LOAD instead of two, and none mid-stream).  Keys/order
    preserved so act_func_set_id indexing is unchanged."""
    t = _orig_gat(arch)
    out = {}
    for k, v in t.items():
        if k == "sqrt_and_others":
            out[k] = v
        else:
            out[k] = {f for f in v if f not in (_AF.Sqrt, _AF.Relu, _AF.Copy)}
    return out


bacc.get_activation_tables = _sqrt_set_only

F32 = mybir.dt.float32
BF16 = mybir.dt.bfloat16
AF = mybir.ActivationFunctionType
OP = mybir.AluOpType

N = 512          # rows
D = 256          # embedding dim
NCORES = 8
A = N // NCORES  # anchors per core
MARGIN = 5.0
EPS = 1e-16
BIGM = 512.0     # same-label mask (power of two, exact in bf16)
DEAD = -1e30     # dead pair-slot kill value

_cache = {}


def _build(Tp: int):
    """Build the (uniform, SPMD) per-core Bass program for Tp pair tiles."""
    tile.TileContext._drain_and_barrier = _cheap_drain_and_barrier
    bass.Bass.all_engine_barrier = _skip_const_barrier
    nc = bacc.Bacc(None, target_bir_lowering=False)
    # gpsimd issues no DMAs in this kernel: drop its queue group so the
    # runtime has fewer rings to manage.
    nc.m.queues = [q for q in nc.m.queues if not q.name.startswith("qPoolDynamic")]

    P = Tp * 128
    # bf16 column layout of the packed input (one DRAM tensor):
    O_REPT = 0                  # [128, 2*512]  rept[p, c*512+j] = rep[j, c*128+p]
    O_REPAT2 = O_REPT + 1024    # [128, 2*64]   -2 * rep[base+a, c*128+p]
    O_PM = O_REPAT2 + 128       # [128, Tp]     MARGIN (live) / DEAD pair slots
    O_AP = O_PM + Tp            # [128, Tp*256] rep[a_{t*128+r}, c] pair-major
    O_PP = O_AP + 2 * P         # [128, Tp*256] rep[p_{t*128+r}, c]
    O_SEL = O_PP + 2 * P        # [64, Tp*128]  one-hot pair->anchor gather
    O_BIGM = O_SEL + P          # [64, 512]     BIGM * same_label
    O_REPA = O_BIGM + 512       # [64, 256]     rep[base+a, :] (row-major)
    COLS = O_REPA + 256

    W = 2 * Tp                  # out cols: [S_0..S_{Tp-1}, C_0..C_{Tp-1}]

    pk_d = nc.declare_dram_parameter("pk", [128, COLS], BF16, isOutput=False)
    out_d = nc.declare_dram_parameter("out", [1, W], F32, isOutput=True)

    with tile.TileContext(nc) as tc:
        with (
            tc.tile_pool(name="singles", bufs=1) as sg,
            tc.tile_pool(name="ppd", bufs=1, space="PSUM") as ppd,
            tc.tile_pool(name="ppg", bufs=4, space="PSUM") as ppg,
            tc.tile_pool(name="ppf", bufs=1, space="PSUM") as ppf,
        ):
            pk = sg.tile([128, COLS], BF16)
            # input loads: sync carries the d2-path inputs then the
            # 64-partition sel/bigm/repa block; scalar carries AP/PP.
            nc.sync.dma_start(pk[:, O_REPT:O_AP], pk_d[:, O_REPT:O_AP])
            nc.scalar.dma_start(pk[:, O_AP:O_SEL], pk_d[:, O_AP:O_SEL])
            nc.sync.dma_start(pk[0:64, O_SEL:COLS], pk_d[0:64, O_SEL:COLS])

            ones_c = sg.tile([128, A], BF16)
            nc.vector.memset(ones_c[:], 1.0)
            onesf = sg.tile([128, 1], F32)
            nc.gpsimd.memset(onesf[:], 1.0)
            wrm = sg.tile([1, 1], F32)
            nc.gpsimd.memset(wrm[:], 1.0)
            # dependency-free warmup: pins the single ACT_TABLE_LOAD to
            # the head of the ACT stream, overlapping the input DMAs
            dmy = sg.tile([1, 1], F32)
            nc.scalar.activation(dmy[:], wrm[:], AF.Sqrt)

            # ---- main d2: sq_a + sq_j - 2 a.j for 64 anchors x 512 j ----
            sqsq = sg.tile([128, 1024], BF16)
            nc.vector.tensor_mul(
                sqsq[:], pk[:, O_REPT:O_REPAT2], pk[:, O_REPT:O_REPAT2]
            )
            d2_p = ppd.tile([A, N], F32, tag="d2")
            for c in range(2):
                nc.tensor.matmul(
                    d2_p[:],
                    pk[:, O_REPAT2 + c * A:O_REPAT2 + (c + 1) * A],
                    pk[:, O_REPT + c * 512:O_REPT + (c + 1) * 512],
                    start=(c == 0), stop=False, skip_group_check=True,
                )
            for c in range(2):
                nc.tensor.matmul(
                    d2_p[:], ones_c[:], sqsq[:, c * 512:(c + 1) * 512],
                    start=False, stop=(c == 1), skip_group_check=True,
                )

            # sq_anch[64,1] (repa rows live on partitions 0-63)
            sqa_scr = sg.tile([64, D], BF16)
            sqanch = sg.tile([A, 1], F32)
            nc.vector.scalar_tensor_tensor(
                out=sqa_scr[:], in0=pk[0:64, O_REPA:O_REPA + 256], scalar=1.0,
                in1=pk[0:64, O_REPA:O_REPA + 256],
                op0=OP.mult, op1=OP.mult, accum_out=sqanch[:],
            )
            sqanchb = sg.tile([A, 1], F32)
            nc.vector.tensor_scalar(sqanchb[:], sqanch[:], 0.25, None, OP.add)

            # ym = sqrt(d2 + 0.25) + BIGM*same  (the +0.25 keeps the masked
            # diagonal's accumulation-order noise out of sqrt's domain)
            dtmp = sg.tile([A, N], BF16)
            nc.scalar.activation(dtmp[:], d2_p[:], AF.Sqrt, bias=sqanchb[:])
            ym = sg.tile([A, N], BF16)
            nc.vector.tensor_add(ym[:], pk[0:64, O_BIGM:O_BIGM + 512], dtmp[:])

            # ---- pair distances: xp = sqrt(sum_c (AP-PP)^2) + pm ----
            diff = sg.tile([128, 2 * P], BF16)
            nc.vector.tensor_tensor(
                diff[:], pk[:, O_AP:O_AP + 2 * P], pk[:, O_PP:O_PP + 2 * P],
                op=OP.subtract,
            )
            dsq = sg.tile([128, D], BF16)
            xpacc = sg.tile([128, Tp], F32)
            for t in range(Tp):
                nc.vector.scalar_tensor_tensor(
                    out=dsq[:], in0=diff[:, t * D:(t + 1) * D], scalar=1.0,
                    in1=diff[:, t * D:(t + 1) * D],
                    op0=OP.mult, op1=OP.mult, accum_out=xpacc[:, t:t + 1],
                )
            xps = sg.tile([128, Tp], F32)
            nc.scalar.activation(xps[:], xpacc[:], AF.Sqrt)
            xp_all = sg.tile([128, Tp], F32)
            nc.vector.tensor_add(xp_all[:], xps[:], pk[:, O_PM:O_PM + Tp])

            # ---- pair tiles: gather, relu-accum (S), count (C) ----
            SC = sg.tile([128, W], F32)
            relbig = sg.tile([128, N], BF16)
            junk = sg.tile([128, N], BF16)
            for t in range(Tp):
                gy = ppg.tile([128, N], F32, tag="gy")
                nc.tensor.matmul(
                    gy[:], pk[0:64, O_SEL + t * 128:O_SEL + (t + 1) * 128],
                    ym[:], start=True, stop=True,
                )
                nc.scalar.activation(
                    relbig[:], gy[:], AF.Relu,
                    bias=xp_all[:, t:t + 1], scale=-1.0,
                    accum_out=SC[:, t:t + 1],
                )
                nc.vector.tensor_scalar(
                    junk[:], gy[:], xp_all[:, t:t + 1], 0.0, OP.is_lt, OP.add,
                    accum_out=SC[:, Tp + t:Tp + t + 1],
                )

            # partition-sum the S and C columns -> [1, W]
            fin_p = ppf.tile([1, W], F32, tag="fin")
            nc.tensor.matmul(fin_p[:], onesf[:], SC[:], start=True, stop=True)
            outsb = sg.tile([1, W], F32)
            nc.vector.tensor_copy(outsb[:], fin_p[:])
            nc.sync.dma_start(out_d[:], outsb[:])

    nc.finalize()
    return nc


def _prep(rep: np.ndarray, labels: np.ndarray):
    """Host-side layout/gather/cast prep: shard anchors, enumerate pairs."""
    rep = np.ascontiguousarray(np.asarray(rep, dtype=np.float32))
    repb = rep.astype(ml_dtypes.bfloat16)
    labels = np.asarray(labels)
    same = labels[:, None] == labels[None, :]

    # rep.T packed: rept[p, c*512 + j] = rep[j, c*128 + p]
    rept = np.ascontiguousarray(
        repb.T.reshape(2, 128, N).transpose(1, 0, 2).reshape(128, 1024)
    )

    pairs = []
    for c in range(NCORES):
        base = c * A
        prs = [
            (j, p)
            for j in range(A)
            for p in np.nonzero(same[base + j])[0]
            if p != base + j
        ]
        pairs.append(prs)
    Tp = max(1, max((len(p) + 127) // 128 for p in pairs))
    P = Tp * 128

    O_REPT = 0
    O_REPAT2 = 1024
    O_PM = O_REPAT2 + 128
    O_AP = O_PM + Tp
    O_PP = O_AP + 2 * P
    O_SEL = O_PP + 2 * P
    O_BIGM = O_SEL + P
    O_REPA = O_BIGM + 512
    COLS = O_REPA + 256

    in_maps = []
    for c in range(NCORES):
        base = c * A
        npair = len(pairs[c])
        a_idx = np.zeros(P, np.int64)
        p_idx = np.zeros(P, np.int64)
        for i, (j, p) in enumerate(pairs[c]):
            a_idx[i] = base + j
            p_idx[i] = p

        pk = np.zeros((128, COLS), ml_dtypes.bfloat16)
        pk[:, O_REPT:O_REPAT2] = rept
        # -2 * anchor transpose (exact scale)
        repa32 = rep[base:base + A]
        pk[:, O_REPAT2:O_PM] = np.ascontiguousarray(
            (-2.0 * repa32).T.reshape(2, 128, A).transpose(1, 0, 2).reshape(128, 2 * A)
        ).astype(ml_dtypes.bfloat16)
        # pair-slot margin / dead-kill values, [128, Tp] pair-major
        pm = np.full(P, DEAD, np.float32)
        pm[:npair] = MARGIN
        pk[:, O_PM:O_AP] = (
            pm.reshape(Tp, 128).T.astype(ml_dtypes.bfloat16)
        )
        # gathered anchor / positive rows (bf16 of rep, pure gather),
        # pair-major: row r, cols [t*256, (t+1)*256) = rep[idx[t*128+r], :]
        gA = repb[a_idx].reshape(Tp, 128, D).transpose(1, 0, 2).reshape(128, 2 * P)
        gP = repb[p_idx].reshape(Tp, 128, D).transpose(1, 0, 2).reshape(128, 2 * P)
        pk[:, O_AP:O_PP] = gA
        pk[:, O_PP:O_SEL] = gP
        # dead AP/PP slots are rep[0] - rep[0] = 0 -> xp = 0 + DEAD
        sel = np.zeros((A, P), ml_dtypes.bfloat16)
        for i, (j, p) in enumerate(pairs[c]):
            sel[j, i] = 1.0
        pk[0:64, O_SEL:O_BIGM] = sel
        pk[0:64, O_BIGM:O_REPA] = np.where(
            same[base:base + A], BIGM, 0.0
        ).astype(ml_dtypes.bfloat16)
        pk[0:64, O_REPA:COLS] = repb[base:base + A]
        in_maps.append({"pk": pk})
    return Tp, in_maps


def _run(rep, labels, trace=False):
    Tp, in_maps = _prep(rep, labels)
    if Tp not in _cache:
        _cache[Tp] = _build(Tp)
    nc = _cache[Tp]
    res = run_bass_kernel_spmd(nc, in_maps, list(range(NCORES)), trace=trace)
    outs = np.stack([res.results[c]["out"][0] for c in range(NCORES)])  # [8, 2Tp]
    S = float(outs[:, 0:Tp].sum())
    C = float(outs[:, Tp:].sum())
    loss = np.float32(S / (C + EPS))
    return np.asarray(loss, dtype=np.float32), res


def kernel(rep, labels):
    loss, _ = _run(rep, labels, trace=False)
    return loss


# revision 19
# speedup vs baseline: 1.3022x; 1.0331x over previous
"""BatchAllTripletLoss kernel for 8 Trainium2 NeuronCores.

Reference computation:
    pd = pairwise_euclidean(rep)                        # [512, 512]
    tl[a,p,k] = relu(pd[a,p] - pd[a,k] + 5.0) * mask    # [512, 512, 512]
    loss = sum(tl) / (count(tl > eps) + eps)

The mask (p!=a, k!=a, p!=k, label[p]==label[a], label[k]!=label[a])
collapses: valid triplets are exactly (anchor-positive pairs) x (k with a
different label).  With 64 labels over 512 rows there are only ~4500
(a,p) pairs, so each core processes its 64 anchors' pairs as rows of
[128-pair, 512-k] tiles:

  per core:
    d[64,512]  = sqrt(d2-matmul-group)                  PE + ACT
    ym         = d + BIGM*same_label                    DVE (bf16)
    xp[128,Tp] = sqrt(sum_c (rep_a - rep_p)^2) + m      DVE diff + per-tile
                 square-accumulate, one ACT sqrt        (host-gathered
                 anchor/positive rows AP/PP, pair-major layout)
    per tile:  gy = sel_t.T @ ym (PE one-hot row gather)
               S_t = accum relu(xp - gy)                ACT (bf16 out)
               C_t = accum (gy < xp)                    DVE on gy
    out[1,10]  = ones.T @ [S_t | C_t]                   PE partition sum

All device data is bf16 (fp32 accumulation in PSUM / accumulators); the
rounding is mean-zero across ~1M triplets and lands ~1e-3 relative on
the loss, well inside the 2e-2 gate.  BIGM=512 masks same-label columns
(xp <= ~50 << 512).  Dead pair slots get xp = -1e30 via the host pmadd
row.  Host-side prep is layout/gather/cast only (plus an exact *-2 on
the anchor transpose); all float arithmetic runs on device.  Anchors
are block-sharded 64 per core; the 8 partial (S, C) pairs are reduced
on the host (the all-reduce of the sharding hint).

Overhead trims: the stock Tile exit protocol is replaced with a
sequencer-drain + two barriers + one semaphore RANGE_CLEAR; gpsimd
issues no DMAs so the qPoolDynamic queue group is pruned from the NEFF;
the activation-table chooser is steered so Sqrt and Relu share one
table load, and a dependency-free warmup sqrt pins that load to the
very start of the ACT stream (off the critical path).
"""

import ml_dtypes
import numpy as np

import concourse.bass as bass
import concourse.tile as tile
from concourse import bacc, mybir
from concourse.bass_utils import run_bass_kernel_spmd
from concourse.vector_clock import ScopedClock


_orig_aeb = bass.Bass.all_engine_barrier


def _skip_const_barrier(self, *, sem_only=False):
    if not getattr(self, "_aeb_skipped_once", False):
        self._aeb_skipped_once = True
        return
    return _orig_aeb(self, sem_only=sem_only)


def _cheap_drain_and_barrier(self, tick_clock, wait_clock):
    """Exit protocol with sequencer-only barriers: the SP drain already
    waits out every engine/DMA tick of the tile clock, so the per-engine
    pipeline drains of the stock double butterfly are redundant here."""
    drain_inst = self.nc.sync.drain()
    wait_clock.add_sem_waits(
        drain_inst.ins, ScopedClock({None: tick_clock.global_clock})
    )
    self.nc.all_engine_barrier(sem_only=True)
    popped = self.nc._tile_sem_poison_stack.pop()
    assert popped is self._sem_poison
    self.nc.clear_and_free_semaphores(list(self.sems.allocated().values()))
    self.nc.all_engine_barrier(sem_only=True)


_orig_gat = bacc.get_activation_tables
_AF = mybir.ActivationFunctionType


def _sqrt_set_only(arch):
    """Strip the functions this kernel uses from every set but
    sqrt_and_others so the table chooser lands them all on one set (one
    ACT_TABLE_LOAD instead of two, and none mid-stream).  Keys/order
    preserved so act_func_set_id indexing is unchanged."""
    t = _orig_gat(arch)
    strip = (_AF.Sqrt, _AF.Relu, _AF.Copy, _AF.Square)
    out = {}
    for k, v in t.items():
        if k == "sqrt_and_others":
            out[k] = v
        else:
            out[k] = {f for f in v if f not in strip}
    return out


bacc.get_activation_tables = _sqrt_set_only

F32 = mybir.dt.float32
BF16 = mybir.dt.bfloat16
AF = mybir.ActivationFunctionType
OP = mybir.AluOpType

N = 512          # rows
D = 256          # embedding dim
NCORES = 8
A = N // NCORES  # anchors per core
MARGIN = 5.0
EPS = 1e-16
BIGM = 512.0     # same-label mask (power of two, exact in bf16)
DEAD = -1e30     # dead pair-slot kill value

_cache = {}


def _build(Tp: int):
    """Build the (uniform, SPMD) per-core Bass program for Tp pair tiles."""
    tile.TileContext._drain_and_barrier = _cheap_drain_and_barrier
    bass.Bass.all_engine_barrier = _skip_const_barrier
    nc = bacc.Bacc(None, target_bir_lowering=False)
    # All DMAs ride the sync HWDGE queue: drop the gpsimd and scalar
    # queue groups so the runtime has fewer rings to manage.
    nc.m.queues = [
        q for q in nc.m.queues
        if not q.name.startswith(("qPoolDynamic", "qActDynamicHW"))
    ]

    P = Tp * 128
    # bf16 column layout of the packed input (one DRAM tensor).  The pair
    # blocks [AP_t | PP_t] are interleaved with the other inputs in DMA
    # priority order: the SDMA engines drain one queue's work mostly
    # sequentially, so issue order on a single queue == landing order.
    O_REPT = 0                  # [128, 2*512]  rept[p, c*512+j] = rep[j, c*128+p]
    O_REPAT2 = O_REPT + 1024    # [128, 2*64]   -2 * rep[base+a, c*128+p]
    O_PM = O_REPAT2 + 128       # [128, Tp]     MARGIN (live) / DEAD pair slots
    O_B = O_PM + Tp             # [128, Tp*512] per tile t: rep[a_.,:] | rep[p_.,:]
    O_SEL = O_B + 4 * P         # [64, Tp*128]  one-hot pair->anchor gather
    O_BIGM = O_SEL + P          # [64, 512]     BIGM * same_label
    O_REPA = O_BIGM + 512       # [64, 256]     rep[base+a, :] (row-major)
    COLS = O_REPA + 256

    W = 2 * Tp                  # out cols: [S_0..S_{Tp-1}, C_0..C_{Tp-1}]

    pk_d = nc.declare_dram_parameter("pk", [128, COLS], BF16, isOutput=False)
    out_d = nc.declare_dram_parameter("out", [1, W], F32, isOutput=True)

    h1 = (Tp + 1) // 2          # pair-block DMA split

    with tile.TileContext(nc) as tc:
        with (
            tc.tile_pool(name="singles", bufs=1) as sg,
            tc.tile_pool(name="ppd", bufs=1, space="PSUM") as ppd,
            tc.tile_pool(name="ppg", bufs=4, space="PSUM") as ppg,
            tc.tile_pool(name="ppf", bufs=1, space="PSUM") as ppf,
        ):
            pk = sg.tile([128, COLS], BF16)
            # input loads, all on the sync queue in landing-priority order:
            # d2 inputs, first pair half, sel/bigm/repa, second pair half.
            nc.sync.dma_start(pk[:, O_REPT:O_B], pk_d[:, O_REPT:O_B])
            nc.sync.dma_start(
                pk[:, O_B:O_B + h1 * 512], pk_d[:, O_B:O_B + h1 * 512]
            )
            nc.sync.dma_start(pk[0:64, O_SEL:COLS], pk_d[0:64, O_SEL:COLS])
            nc.sync.dma_start(
                pk[:, O_B + h1 * 512:O_SEL], pk_d[:, O_B + h1 * 512:O_SEL]
            )

            ones_c = sg.tile([128, A], BF16)
            nc.vector.memset(ones_c[:], 1.0)
            onesf = sg.tile([128, 1], F32)
            nc.gpsimd.memset(onesf[:], 1.0)
            wrm = sg.tile([1, 1], F32)
            nc.gpsimd.memset(wrm[:], 1.0)
            # dependency-free warmup: pins the single ACT_TABLE_LOAD to
            # the head of the ACT stream, overlapping the input DMAs
            dmy = sg.tile([1, 1], F32)
            nc.scalar.activation(dmy[:], wrm[:], AF.Sqrt)

            # ---- main d2: sq_a + sq_j - 2 a.j for 64 anchors x 512 j ----
            sqsq = sg.tile([128, 1024], BF16)
            nc.vector.tensor_mul(
                sqsq[:], pk[:, O_REPT:O_REPAT2], pk[:, O_REPT:O_REPAT2]
            )
            d2_p = ppd.tile([A, N], F32, tag="d2")
            for c in range(2):
                nc.tensor.matmul(
                    d2_p[:],
                    pk[:, O_REPAT2 + c * A:O_REPAT2 + (c + 1) * A],
                    pk[:, O_REPT + c * 512:O_REPT + (c + 1) * 512],
                    start=(c == 0), stop=False, skip_group_check=True,
                )
            for c in range(2):
                nc.tensor.matmul(
                    d2_p[:], ones_c[:], sqsq[:, c * 512:(c + 1) * 512],
                    start=False, stop=(c == 1), skip_group_check=True,
                )

            # sq_anch[64,1] on ACT (repa rows live on partitions 0-63)
            sqa_scr = sg.tile([64, D], BF16)
            sqanch = sg.tile([A, 1], F32)
            nc.scalar.activation(
                sqa_scr[:], pk[0:64, O_REPA:O_REPA + 256], AF.Square,
                accum_out=sqanch[:],
            )
            sqanchb = sg.tile([A, 1], F32)
            nc.vector.tensor_scalar(sqanchb[:], sqanch[:], 0.25, None, OP.add)

            # ym = sqrt(d2 + 0.25) + BIGM*same  (the +0.25 keeps the masked
            # diagonal's accumulation-order noise out of sqrt's domain)
            dtmp = sg.tile([A, N], BF16)
            nc.scalar.activation(dtmp[:], d2_p[:], AF.Sqrt, bias=sqanchb[:])
            ym = sg.tile([A, N], BF16)
            nc.vector.tensor_add(ym[:], pk[0:64, O_BIGM:O_BIGM + 512], dtmp[:])

            # ---- pair distances: xp = sqrt(sum_c (AP-PP)^2) + pm ----
            diff = sg.tile([128, D], BF16)
            dsq = sg.tile([128, D], BF16)
            xpacc = sg.tile([128, Tp], F32)
            for t in range(Tp):
                bt = O_B + t * 512
                nc.vector.tensor_tensor(
                    diff[:], pk[:, bt:bt + D], pk[:, bt + D:bt + 512],
                    op=OP.subtract,
                )
                nc.vector.scalar_tensor_tensor(
                    out=dsq[:], in0=diff[:], scalar=1.0, in1=diff[:],
                    op0=OP.mult, op1=OP.mult, accum_out=xpacc[:, t:t + 1],
                )
            xps = sg.tile([128, Tp], F32)
            nc.scalar.activation(xps[:], xpacc[:], AF.Sqrt)
            xp_all = sg.tile([128, Tp], F32)
            nc.vector.tensor_add(xp_all[:], xps[:], pk[:, O_PM:O_PM + Tp])

            # ---- pair tiles: gather, relu-accum (S), count (C) ----
            SC = sg.tile([128, W], F32)
            relbig = sg.tile([128, N], BF16)
            junk = sg.tile([128, N], BF16)
            for t in range(Tp):
                gy = ppg.tile([128, N], F32, tag="gy")
                nc.tensor.matmul(
                    gy[:], pk[0:64, O_SEL + t * 128:O_SEL + (t + 1) * 128],
                    ym[:], start=True, stop=True,
                )
                nc.scalar.activation(
                    relbig[:], gy[:], AF.Relu,
                    bias=xp_all[:, t:t + 1], scale=-1.0,
                    accum_out=SC[:, t:t + 1],
                )
                nc.vector.tensor_scalar(
                    junk[:], gy[:], xp_all[:, t:t + 1], 0.0, OP.is_lt, OP.add,
                    accum_out=SC[:, Tp + t:Tp + t + 1],
                )

            # partition-sum the S and C columns -> [1, W]
            fin_p = ppf.tile([1, W], F32, tag="fin")
            nc.tensor.matmul(fin_p[:], onesf[:], SC[:], start=True, stop=True)
            outsb = sg.tile([1, W], F32)
            nc.vector.tensor_copy(outsb[:], fin_p[:])
            nc.sync.dma_start(out_d[:], outsb[:])

    nc.finalize()
    return nc


def _prep(rep: np.ndarray, labels: np.ndarray):
    """Host-side layout/gather/cast prep: shard anchors, enumerate pairs."""
    rep = np.ascontiguousarray(np.asarray(rep, dtype=np.float32))
    repb = rep.astype(ml_dtypes.bfloat16)
    labels = np.asarray(labels)
    same = labels[:, None] == labels[None, :]

    # rep.T packed: rept[p, c*512 + j] = rep[j, c*128 + p]
    rept = np.ascontiguousarray(
        repb.T.reshape(2, 128, N).transpose(1, 0, 2).reshape(128, 1024)
    )

    pairs = []
    for c in range(NCORES):
        base = c * A
        prs = [
            (j, p)
            for j in range(A)
            for p in np.nonzero(same[base + j])[0]
            if p != base + j
        ]
        pairs.append(prs)
    Tp = max(1, max((len(p) + 127) // 128 for p in pairs))
    P = Tp * 128

    O_REPT = 0
    O_REPAT2 = 1024
    O_PM = O_REPAT2 + 128
    O_B = O_PM + Tp
    O_SEL = O_B + 4 * P
    O_BIGM = O_SEL + P
    O_REPA = O_BIGM + 512
    COLS = O_REPA + 256

    in_maps = []
    for c in range(NCORES):
        base = c * A
        npair = len(pairs[c])
        a_idx = np.zeros(P, np.int64)
        p_idx = np.zeros(P, np.int64)
        for i, (j, p) in enumerate(pairs[c]):
            a_idx[i] = base + j
            p_idx[i] = p

        pk = np.zeros((128, COLS), ml_dtypes.bfloat16)
        pk[:, O_REPT:O_REPAT2] = rept
        # -2 * anchor transpose (exact scale)
        repa32 = rep[base:base + A]
        pk[:, O_REPAT2:O_PM] = np.ascontiguousarray(
            (-2.0 * repa32).T.reshape(2, 128, A).transpose(1, 0, 2).reshape(128, 2 * A)
        ).astype(ml_dtypes.bfloat16)
        # pair-slot margin / dead-kill values, [128, Tp] pair-major
        pm = np.full(P, DEAD, np.float32)
        pm[:npair] = MARGIN
        pk[:, O_PM:O_B] = (
            pm.reshape(Tp, 128).T.astype(ml_dtypes.bfloat16)
        )
        # gathered anchor / positive rows (bf16 of rep, pure gather),
        # per-tile blocks: row r, cols [t*512, t*512+256) = rep[a_{t*128+r}]
        # and [t*512+256, (t+1)*512) = rep[p_{t*128+r}]
        gA = repb[a_idx].reshape(Tp, 128, D).transpose(1, 0, 2)  # [128, Tp, D]
        gP = repb[p_idx].reshape(Tp, 128, D).transpose(1, 0, 2)
        blk = np.concatenate([gA, gP], axis=2)                   # [128, Tp, 2D]
        pk[:, O_B:O_SEL] = blk.reshape(128, 4 * P)
        # dead AP/PP slots are rep[0] - rep[0] = 0 -> xp = 0 + DEAD
        sel = np.zeros((A, P), ml_dtypes.bfloat16)
        for i, (j, p) in enumerate(pairs[c]):
            sel[j, i] = 1.0
        pk[0:64, O_SEL:O_BIGM] = sel
        pk[0:64, O_BIGM:O_REPA] = np.where(
            same[base:base + A], BIGM, 0.0
        ).astype(ml_dtypes.bfloat16)
        pk[0:64, O_REPA:COLS] = repb[base:base + A]
        in_maps.append({"pk": pk})
    return Tp, in_maps


def _run(rep, labels, trace=False):
    Tp, in_maps = _prep(rep, labels)
    if Tp not in _cache:
        _cache[Tp] = _build(Tp)
    nc = _cache[Tp]
    res = run_bass_kernel_spmd(nc, in_maps, list(range(NCORES)), trace=trace)
    outs = np.stack([res.results[c]["out"][0] for c in range(NCORES)])  # [8, 2Tp]
    S = float(outs[:, 0:Tp].sum())
    C = float(outs[:, Tp:].sum())
    loss = np.float32(S / (C + EPS))
    return np.asarray(loss, dtype=np.float32), res


def kernel(rep, labels):
    loss, _ = _run(rep, labels, trace=False)
    return loss
